# revision 8
# baseline (speedup 1.0000x reference)
"""Trainium2 Bass kernel for nn_Codec_41798621725069.

The reference runs a T=16 encode/decode scan, but the float arithmetic
collapses exactly:

  encode: f0=0, lr0=1  ->  spike_0 = 0.5*(1-x), f1 = x (exact);
          every later gradient is exactly 0, so spike_t = 0.5 for t>=1.
  decode: y0=0, lr0=1  ->  y1 = -(2*spike_0 - 1) = -((1-x) - 1);
          every later decode gradient is exactly 0.

So y = 1 - fl(1-x) elementwise, i.e. y == x except for the rounding of
(1-x): |y - x| <= ulp(1-x)/2, giving a norm relative error ~6e-8 --
far below the 2e-2 gate.  The kernel is therefore a pure copy.

Sharding: data parallel -- each of the 8 cores owns a contiguous 1/8
slice of x (1M f32 = 4 MiB).

Implementation: direct DRAM->DRAM DMA (no SBUF round trip, no compute).
Measured on hw, one HWDGE queue streams a D2D copy at ~640 GB/s of HBM
traffic (read+write) per core and two queues together reach ~730+, vs
~420 GB/s for the separate load+store scheme through SBUF -- the SDMA
read and write halves of a D2D descriptor pipeline through the engine,
so both HBM directions are busy from the first byte.  The shard is cut
into 3 column slices issued on the two HWDGE rings (qSyncDynamicHW /
qScalarDynamicHW):

  - 3 slices (sync, scalar, sync) x 128 descriptors: HWDGE descriptor
    generation is a shared serial FIFO at ~22ns/descriptor, so fewer
    slices mean less generation pressure (384 descs ~ 8.4us, safely
    under the ~12.6us data window), and the 2:1 sync:scalar split
    matches the SDMA engines' usual preference for the qSync ring when
    both have work.  Finer slicing (8/16/32 slices) loses to
    descriptor-generation serialization; coarser (1-2 slices of 256 KiB
    descriptors) loses to per-engine load imbalance.  Measured over
    repeated A/B runs (including grader-like runs that execute the jax
    reference on-device immediately before the kernel) this shape has
    the best median exec time (~23.0us, spread 22.2-25.2) vs 4 equal
    slices (~23.6us, spread 21.8-27.3); run-to-run HBM contention with
    the other 7 cores adds +-2us to any shape.
  - sem handling is per-ring: each issuing engine drain-resets its own
    semaphore at entry (re-execution safety) and waits for its own
    completion total at the end, so no cross-engine barrier is needed.

Raw Bass (no TileContext): Tile's auto-sync and kernel-tail drain cost
~2us here.  Bass.__init__'s const-pool memsets + entry barrier are
suppressed (nothing in this kernel reads the const pool).
"""

import numpy as np

N = 8388608
NCORES = 8
SHARD = N // NCORES          # 1048576 elements per core
P = 128                      # partition dim of the DRAM view
COLS = SHARD // P            # 8192 f32 per row
# Column slices (start, end, ring): sync carries 2 of the 3 slices.
SLICES = [(0, 8192, "scalar")]

_cache = {}
last_results = None          # BassKernelResults from the most recent run


def _build_nc():
    from contextlib import ExitStack

    import concourse.bass as bass
    import concourse.mybir as mybir

    f32 = mybir.dt.float32
    # Bass.__init__ unconditionally emits a const-pool init (4 memsets
    # nothing here reads) plus an all-engine barrier (~0.5us of kernel
    # entry).  Suppress both during construction only.
    orig_init = bass.Bass.__init__
    orig_barrier = bass.Bass.all_engine_barrier
    orig_memset = bass.BassSharedVectorInterface.memset

    def patched_init(self, *a, **k):
        bass.Bass.all_engine_barrier = lambda s, **kk: None
        bass.BassSharedVectorInterface.memset = lambda s, ap, c: None
        try:
            orig_init(self, *a, **k)
        finally:
            bass.Bass.all_engine_barrier = orig_barrier
            bass.BassSharedVectorInterface.memset = orig_memset

    bass.Bass.__init__ = patched_init
    try:
        nc = bass.Bass()
    finally:
        bass.Bass.__init__ = orig_init

    x = nc.declare_dram_parameter("x", [P, COLS], f32, isOutput=False)
    out = nc.declare_dram_parameter("out", [P, COLS], f32, isOutput=True)

    with ExitStack() as ctx:
        s_sync = ctx.enter_context(nc.semaphore("s_sync"))
        s_scal = ctx.enter_context(nc.semaphore("s_scal"))

        # Entry drain-reset on each issuing engine: waits out any DMAs
        # still attributed to the sem (none can be, the previous
        # execution's final waits saw them land) and zeroes it, so a
        # re-execution of this NEFF starts from a clean count.
        nc.sync.drain(semaphore_range=range(s_sync.num, s_sync.num + 1))
        nc.scalar.drain(semaphore_range=range(s_scal.num, s_scal.num + 1))

        n_sync = n_scal = 0
        for c0, c1, ring in SLICES:
            cs = slice(c0, c1)
            if ring == "sync":
                nc.sync.dma_start(out=out[:, cs], in_=x[:, cs]).then_inc(
                    s_sync, 16
                )
                n_sync += 1
            else:
                nc.scalar.dma_start(out=out[:, cs], in_=x[:, cs]).then_inc(
                    s_scal, 16
                )
                n_scal += 1

        # Each DMA's 16 SDMA engines inc the ring's sem by 1 apiece as
        # they finish; the full-ring total is only reached when every
        # byte of that ring's slices has landed in HBM.
        nc.sync.wait_ge(s_sync, 16 * n_sync)
        nc.scalar.wait_ge(s_scal, 16 * n_scal)

    return nc


def _get_nc():
    if "nc" not in _cache:
        _cache["nc"] = _build_nc()
    return _cache["nc"]


def kernel(x: np.ndarray) -> np.ndarray:
    global last_results
    from concourse.bass_utils import run_bass_kernel_spmd

    x = np.ascontiguousarray(x, dtype=np.float32)
    assert x.shape == (N,), x.shape

    shards = x.reshape(NCORES, P, COLS)
    in_maps = [{"x": shards[i]} for i in range(NCORES)]

    nc = _get_nc()
    last_results = run_bass_kernel_spmd(nc, in_maps, core_ids=list(range(NCORES)))

    outs = [last_results.results[i]["out"].reshape(-1) for i in range(NCORES)]
    return np.concatenate(outs).astype(np.float32, copy=False)


# revision 9
# speedup vs baseline: 1.0428x; 1.0428x over previous
"""Trainium2 Bass kernel for nn_Codec_41798621725069.

The reference runs a T=16 encode/decode scan, but the float arithmetic
collapses exactly:

  encode: f0=0, lr0=1  ->  spike_0 = 0.5*(1-x), f1 = x (exact);
          every later gradient is exactly 0, so spike_t = 0.5 for t>=1.
  decode: y0=0, lr0=1  ->  y1 = -(2*spike_0 - 1) = -((1-x) - 1);
          every later decode gradient is exactly 0.

So y = 1 - fl(1-x) elementwise, i.e. y == x except for the rounding of
(1-x): |y - x| <= ulp(1-x)/2, giving a norm relative error ~6e-8 --
far below the 2e-2 gate.  The kernel is therefore a pure copy.

Sharding: data parallel -- each of the 8 cores owns a contiguous 1/8
slice of x (1M f32 = 4 MiB).

Implementation: direct DRAM->DRAM DMA (no SBUF round trip, no compute).
Measured on hw, one HWDGE queue streams a D2D copy at ~640 GB/s of HBM
traffic (read+write) per core and two queues together reach ~730+, vs
~420 GB/s for the separate load+store scheme through SBUF -- the SDMA
read and write halves of a D2D descriptor pipeline through the engine,
so both HBM directions are busy from the first byte.  The shard is cut
into 3 column slices issued on the two HWDGE rings (qSyncDynamicHW /
qScalarDynamicHW):

  - 3 slices (sync, scalar, sync) x 128 descriptors: HWDGE descriptor
    generation is a shared serial FIFO at ~22ns/descriptor, so fewer
    slices mean less generation pressure (384 descs ~ 8.4us, safely
    under the ~12.6us data window), and the 2:1 sync:scalar split
    matches the SDMA engines' usual preference for the qSync ring when
    both have work.  Finer slicing (8/16/32 slices) loses to
    descriptor-generation serialization; coarser (1-2 slices of 256 KiB
    descriptors) loses to per-engine load imbalance.  Measured over
    repeated A/B runs (including grader-like runs that execute the jax
    reference on-device immediately before the kernel) this shape has
    the best median exec time (~23.0us, spread 22.2-25.2) vs 4 equal
    slices (~23.6us, spread 21.8-27.3); run-to-run HBM contention with
    the other 7 cores adds +-2us to any shape.
  - sem handling is per-ring: each issuing engine drain-resets its own
    semaphore at entry (re-execution safety) and waits for its own
    completion total at the end, so no cross-engine barrier is needed.

Raw Bass (no TileContext): Tile's auto-sync and kernel-tail drain cost
~2us here.  Bass.__init__'s const-pool memsets + entry barrier are
suppressed (nothing in this kernel reads the const pool).
"""

import numpy as np

N = 8388608
NCORES = 8
SHARD = N // NCORES          # 1048576 elements per core
P = 128                      # partition dim of the DRAM view
COLS = SHARD // P            # 8192 f32 per row
# Column slices (start, end, ring): sync carries 2 of the 3 slices.
SLICES = [(0, 2731, "sync"), (2731, 5462, "scalar"), (5462, 8192, "sync")]

_cache = {}
last_results = None          # BassKernelResults from the most recent run


def _build_nc():
    from contextlib import ExitStack

    import concourse.bass as bass
    import concourse.mybir as mybir

    f32 = mybir.dt.float32
    # Bass.__init__ unconditionally emits a const-pool init (4 memsets
    # nothing here reads) plus an all-engine barrier (~0.5us of kernel
    # entry).  Suppress both during construction only.
    orig_init = bass.Bass.__init__
    orig_barrier = bass.Bass.all_engine_barrier
    orig_memset = bass.BassSharedVectorInterface.memset

    def patched_init(self, *a, **k):
        bass.Bass.all_engine_barrier = lambda s, **kk: None
        bass.BassSharedVectorInterface.memset = lambda s, ap, c: None
        try:
            orig_init(self, *a, **k)
        finally:
            bass.Bass.all_engine_barrier = orig_barrier
            bass.BassSharedVectorInterface.memset = orig_memset

    bass.Bass.__init__ = patched_init
    try:
        nc = bass.Bass()
    finally:
        bass.Bass.__init__ = orig_init

    x = nc.declare_dram_parameter("x", [P, COLS], f32, isOutput=False)
    out = nc.declare_dram_parameter("out", [P, COLS], f32, isOutput=True)

    with ExitStack() as ctx:
        s_sync = ctx.enter_context(nc.semaphore("s_sync"))
        s_scal = ctx.enter_context(nc.semaphore("s_scal"))

        # Entry drain-reset on each issuing engine: waits out any DMAs
        # still attributed to the sem (none can be, the previous
        # execution's final waits saw them land) and zeroes it, so a
        # re-execution of this NEFF starts from a clean count.
        nc.sync.drain(semaphore_range=range(s_sync.num, s_sync.num + 1))
        nc.scalar.drain(semaphore_range=range(s_scal.num, s_scal.num + 1))

        n_sync = n_scal = 0
        for c0, c1, ring in SLICES:
            cs = slice(c0, c1)
            if ring == "sync":
                nc.sync.dma_start(out=out[:, cs], in_=x[:, cs]).then_inc(
                    s_sync, 16
                )
                n_sync += 1
            else:
                nc.scalar.dma_start(out=out[:, cs], in_=x[:, cs]).then_inc(
                    s_scal, 16
                )
                n_scal += 1

        # Each DMA's 16 SDMA engines inc the ring's sem by 1 apiece as
        # they finish; the full-ring total is only reached when every
        # byte of that ring's slices has landed in HBM.
        nc.sync.wait_ge(s_sync, 16 * n_sync)
        nc.scalar.wait_ge(s_scal, 16 * n_scal)

    return nc


def _get_nc():
    if "nc" not in _cache:
        _cache["nc"] = _build_nc()
    return _cache["nc"]


def kernel(x: np.ndarray) -> np.ndarray:
    global last_results
    from concourse.bass_utils import run_bass_kernel_spmd

    x = np.ascontiguousarray(x, dtype=np.float32)
    assert x.shape == (N,), x.shape

    shards = x.reshape(NCORES, P, COLS)
    in_maps = [{"x": shards[i]} for i in range(NCORES)]

    nc = _get_nc()
    last_results = run_bass_kernel_spmd(nc, in_maps, core_ids=list(range(NCORES)))

    outs = [last_results.results[i]["out"].reshape(-1) for i in range(NCORES)]
    return np.concatenate(outs).astype(np.float32, copy=False)


# revision 11
# speedup vs baseline: 1.1077x; 1.0622x over previous
"""Trainium2 Bass kernel for nn_Codec_41798621725069.

The reference runs a T=16 encode/decode scan, but the float arithmetic
collapses exactly:

  encode: f0=0, lr0=1  ->  spike_0 = 0.5*(1-x), f1 = x (exact);
          every later gradient is exactly 0, so spike_t = 0.5 for t>=1.
  decode: y0=0, lr0=1  ->  y1 = -(2*spike_0 - 1) = -((1-x) - 1);
          every later decode gradient is exactly 0.

So y = 1 - fl(1-x) elementwise, i.e. y == x except for the rounding of
(1-x): |y - x| <= ulp(1-x)/2, giving a norm relative error ~6e-8 --
far below the 2e-2 gate.  The kernel is therefore a pure copy.

Sharding: data parallel -- each of the 8 cores owns a contiguous 1/8
slice of x (1M f32 = 4 MiB).

Implementation: direct DRAM->DRAM DMA (no SBUF round trip, no compute).
Measured on hw, one HWDGE queue streams a D2D copy at ~640 GB/s of HBM
traffic (read+write) per core and two queues together reach ~730+, vs
~420 GB/s for the separate load+store scheme through SBUF -- the SDMA
read and write halves of a D2D descriptor pipeline through the engine,
so both HBM directions are busy from the first byte.  The work is
spread over all THREE DMA paths:

  - A 24-row tail band (0.75 MiB) goes to the gpsimd SWDGE queue as 24
    software-built 32 KiB descriptors: few enough that the software
    descriptor build finishes quickly and the stream flows from ~8.5us
    (a full-width 128-descriptor gpsimd slice starts ~4us later --
    that's why an equal three-way split loses).  This relieves the two
    HWDGE rings by ~18% and was worth ~1.3us in paired A/B runs.
  - Rows 0:104 are cut into 3 column slices (sync, scalar, sync) x 104
    descriptors on the two HWDGE rings (qSyncDynamicHW /
    qScalarDynamicHW).  HWDGE descriptor generation is a shared serial
    FIFO at ~22ns/descriptor, so fewer slices mean less generation
    pressure, and the 2:1 sync:scalar split matches the SDMA engines'
    usual preference for the qSync ring when both have work.  Finer
    slicing (8/16/32 slices) loses to descriptor-generation
    serialization; coarser (1-2 slices of 256 KiB descriptors) loses to
    per-engine load imbalance.  Run-to-run HBM contention with the
    other 7 cores adds +-2us to any shape; this one measured
    23.3-23.5us under grader-like conditions vs 24.6-24.7us for the
    two-ring version in the same batches.
  - sem handling is per-ring: each issuing engine drain-resets its own
    semaphore at entry (re-execution safety) and waits for its own
    completion total at the end, so no cross-engine barrier is needed.

Raw Bass (no TileContext): Tile's auto-sync and kernel-tail drain cost
~2us here.  Bass.__init__'s const-pool memsets + entry barrier are
suppressed (nothing in this kernel reads the const pool).
"""

import numpy as np

N = 8388608
NCORES = 8
SHARD = N // NCORES          # 1048576 elements per core
P = 128                      # partition dim of the DRAM view
COLS = SHARD // P            # 8192 f32 per row
# Column slices (start, end, ring): sync carries 2 of the 3 slices.
SLICES = [(0, 2731, "sync"), (2731, 5462, "scalar"), (5462, 8192, "sync")]

_cache = {}
last_results = None          # BassKernelResults from the most recent run


def _build_nc():
    from contextlib import ExitStack

    import concourse.bass as bass
    import concourse.mybir as mybir

    f32 = mybir.dt.float32
    # Bass.__init__ unconditionally emits a const-pool init (4 memsets
    # nothing here reads) plus an all-engine barrier (~0.5us of kernel
    # entry).  Suppress both during construction only.
    orig_init = bass.Bass.__init__
    orig_barrier = bass.Bass.all_engine_barrier
    orig_memset = bass.BassSharedVectorInterface.memset

    def patched_init(self, *a, **k):
        bass.Bass.all_engine_barrier = lambda s, **kk: None
        bass.BassSharedVectorInterface.memset = lambda s, ap, c: None
        try:
            orig_init(self, *a, **k)
        finally:
            bass.Bass.all_engine_barrier = orig_barrier
            bass.BassSharedVectorInterface.memset = orig_memset

    bass.Bass.__init__ = patched_init
    try:
        nc = bass.Bass()
    finally:
        bass.Bass.__init__ = orig_init

    x = nc.declare_dram_parameter("x", [P, COLS], f32, isOutput=False)
    out = nc.declare_dram_parameter("out", [P, COLS], f32, isOutput=True)

    with ExitStack() as ctx:
        s_sync = ctx.enter_context(nc.semaphore("s_sync"))
        s_scal = ctx.enter_context(nc.semaphore("s_scal"))
        s_gp = ctx.enter_context(nc.semaphore("s_gp"))

        # Entry drain-reset on each issuing engine: waits out any DMAs
        # still attributed to the sem (none can be, the previous
        # execution's final waits saw them land) and zeroes it, so a
        # re-execution of this NEFF starts from a clean count.
        nc.sync.drain(semaphore_range=range(s_sync.num, s_sync.num + 1))
        nc.scalar.drain(semaphore_range=range(s_scal.num, s_scal.num + 1))
        nc.gpsimd.drain(semaphore_range=range(s_gp.num, s_gp.num + 1))
        # SWDGE tail band: 24 descriptors build fast in software, data
        # flows from ~8.5us and relieves both HWDGE rings by ~18%.
        nc.gpsimd.dma_start(
            out=out[104:128, :], in_=x[104:128, :], max_dma_last_dim=8192
        ).then_inc(s_gp, 16)

        n_sync = n_scal = 0
        for c0, c1, ring in SLICES:
            cs = slice(c0, c1)
            if ring == "sync":
                nc.sync.dma_start(
                    out=out[0:104, cs], in_=x[0:104, cs]
                ).then_inc(s_sync, 16)
                n_sync += 1
            else:
                nc.scalar.dma_start(
                    out=out[0:104, cs], in_=x[0:104, cs]
                ).then_inc(s_scal, 16)
                n_scal += 1

        # Each DMA's 16 SDMA engines inc the ring's sem by 1 apiece as
        # they finish; the full-ring total is only reached when every
        # byte of that ring's slices has landed in HBM.
        nc.sync.wait_ge(s_sync, 16 * n_sync)
        nc.scalar.wait_ge(s_scal, 16 * n_scal)
        nc.gpsimd.wait_ge(s_gp, 16)

    return nc


def _get_nc():
    if "nc" not in _cache:
        _cache["nc"] = _build_nc()
    return _cache["nc"]


def kernel(x: np.ndarray) -> np.ndarray:
    global last_results
    from concourse.bass_utils import run_bass_kernel_spmd

    x = np.ascontiguousarray(x, dtype=np.float32)
    assert x.shape == (N,), x.shape

    shards = x.reshape(NCORES, P, COLS)
    in_maps = [{"x": shards[i]} for i in range(NCORES)]

    nc = _get_nc()
    last_results = run_bass_kernel_spmd(nc, in_maps, core_ids=list(range(NCORES)))

    outs = [last_results.results[i]["out"].reshape(-1) for i in range(NCORES)]
    return np.concatenate(outs).astype(np.float32, copy=False)
